# revision 1
# baseline (speedup 1.0000x reference)
"""Trainium2 Bass kernel for an AttentionBlock (InstanceNorm + single-head
spatial self-attention + projection + residual).

Full-input contract: kernel(**inputs) takes the complete tensors and returns
the complete output. Internally shards across 8 NeuronCores: data-parallel
over batch (B=4 -> 4 pairs of cores), sequence-parallel over the N=4096 query
positions within each sample (2 cores per sample, 2048 queries each).

All 8 cores run the *same* program; the query-half assignment is done by
rotating the spatial columns of x host-side (attention and instance-norm
statistics are invariant under column permutation).
"""

import os
import sys
import numpy as np
from contextlib import ExitStack

for _p in ("/opt/trn_rl_repo", "/root/.axon_site/_ro/trn_rl_repo"):
    if os.path.isdir(_p) and _p not in sys.path:
        sys.path.append(_p)

from concourse import bass, bacc, tile, mybir, masks  # noqa: E402
from concourse.bass_utils import run_bass_kernel_spmd  # noqa: E402

F32 = mybir.dt.float32
F16 = mybir.dt.float16

B, C, H, W = 4, 64, 64, 64
N = H * W            # 4096 spatial positions (attention length)
HALF = N // 2        # queries per core
QT = 128             # query rows per tile
NQT = HALF // QT     # 16 query tiles per core
KC = 512             # score-matmul free-dim chunk (one PSUM bank of fp32)
NKC = N // KC        # 8 chunks per row
GRP = 2              # query tiles per attn@v group
EPS = 1e-5
NCORES = 8



def build_nc():
    nc = bacc.Bacc("TRN2", target_bir_lowering=False, debug=False)

    x_d = nc.dram_tensor("x", [C, N], F32, kind="ExternalInput")
    wq_d = nc.dram_tensor("wq1", [C + 1, 2, C], F32, kind="ExternalInput")
    wk_d = nc.dram_tensor("wk1", [C + 1, 2, C], F32, kind="ExternalInput")
    wv_d = nc.dram_tensor("wv1", [C + 1, C], F32, kind="ExternalInput")
    wo_d = nc.dram_tensor("wo16", [C, C], F16, kind="ExternalInput")
    bo_d = nc.dram_tensor("bo", [C, 1], F32, kind="ExternalInput")
    out_d = nc.dram_tensor("out", [C, HALF], F32, kind="ExternalOutput")
    scr_d = nc.dram_tensor("scr_inv", [NQT, QT], F32)

    with tile.TileContext(nc) as tc:
        _body(tc, x_d, wq_d, wk_d, wv_d, wo_d, bo_d, out_d, scr_d)
    nc.compile()
    return nc


def _body(tc, x_d, wq_d, wk_d, wv_d, wo_d, bo_d, out_d, scr_d):
    nc = tc.nc
    BF16 = mybir.dt.bfloat16
    with ExitStack() as ctx:
        persist = ctx.enter_context(tc.tile_pool(name="persist", bufs=1))
        small = ctx.enter_context(tc.tile_pool(name="small", bufs=6))
        attn_pool = ctx.enter_context(tc.tile_pool(name="attn_pool", bufs=3))
        attnT_pool = ctx.enter_context(tc.tile_pool(name="attnT_pool", bufs=2))
        park_pool = ctx.enter_context(tc.tile_pool(name="park_pool", bufs=3))
        # PSUM: 3 x [128,2,512]f32 = 6 banks for scores; 2 x 2KB generic slots.
        sp = ctx.enter_context(tc.tile_pool(name="sp", bufs=3, space="PSUM"))
        gp = ctx.enter_context(tc.tile_pool(name="gp", bufs=2, space="PSUM"))

        # ---- constants / inputs ----
        x_sb = persist.tile([C, N], F32)
        nc.sync.dma_start(out=x_sb, in_=x_d.ap())
        wq_sb = persist.tile([C + 1, 2, C], F32)
        nc.sync.dma_start(out=wq_sb, in_=wq_d.ap())
        wk_sb = persist.tile([C + 1, 2, C], F32)
        nc.sync.dma_start(out=wk_sb, in_=wk_d.ap())
        # device-side bf16 rounding of the hi/lo halves prepared on host
        wqhl = persist.tile([C + 1, 2, C], BF16)
        nc.vector.tensor_copy(wqhl, wq_sb)
        wkhl = persist.tile([C + 1, 2, C], BF16)
        nc.vector.tensor_copy(wkhl, wk_sb)
        wv_sb = persist.tile([C + 1, C], F32)
        nc.sync.dma_start(out=wv_sb, in_=wv_d.ap())
        wo_sb = persist.tile([C, C], F16)
        nc.sync.dma_start(out=wo_sb, in_=wo_d.ap())
        bo_sb = persist.tile([C, 1], F32)
        nc.sync.dma_start(out=bo_sb, in_=bo_d.ap())
        eps_t = persist.tile([C, 1], F32)
        nc.vector.memset(eps_t, EPS)

        # ---- instance norm: mean/var per channel over all N positions ----
        stats = persist.tile([C, NKC, nc.vector.BN_STATS_DIM], F32)
        for i in range(NKC):
            nc.vector.bn_stats(out=stats[:, i, :], in_=x_sb[:, i * KC:(i + 1) * KC])
        mv = persist.tile([C, nc.vector.BN_AGGR_DIM], F32)
        nc.vector.bn_aggr(out=mv, in_=stats)
        stdv = persist.tile([C, 1], F32)
        nc.scalar.activation(out=stdv, in_=mv[:, 1:2],
                             func=mybir.ActivationFunctionType.Sqrt,
                             bias=eps_t, scale=1.0)
        rstd = persist.tile([C, 1], F32)
        nc.vector.reciprocal(out=rstd, in_=stdv)
        nmr = persist.tile([C, 1], F32)
        nc.vector.tensor_mul(nmr, mv[:, 0:1], rstd)
        nc.vector.tensor_scalar_mul(nmr, nmr, -1.0)

        # xn1[0:C] = normalized x; row C = ones (bias row for QKV matmuls)
        xn1 = persist.tile([C + 1, N], F32)
        for i in range(2):
            hl = slice(i * (N // 2), (i + 1) * (N // 2))
            nc.scalar.activation(out=xn1[0:C, hl], in_=x_sb[:, hl],
                                 func=mybir.ActivationFunctionType.Identity,
                                 bias=nmr, scale=rstd)
        nc.gpsimd.memset(xn1[C:C + 1, :], 1.0)
        # bf16 hi/lo of xn (for bf16-trio QKV matmuls)
        xnh = persist.tile([C + 1, N], BF16)
        xnl = persist.tile([C + 1, N], BF16)
        for i in range(2):
            hl = slice(i * (N // 2), (i + 1) * (N // 2))
            nc.vector.tensor_copy(xnh[:, hl], xn1[:, hl])
            nc.vector.tensor_sub(xnl[:, hl], xn1[:, hl], xnh[:, hl])

        # identity (fp16) for PE-mode transposes
        ident = persist.tile([QT, QT], F16)
        masks.make_identity(nc, ident)

        # ---- QKV projections ----
        # k/q layouts:
        #  PACK2: k2 [128, N/2] holds chunk 2c in partitions 0:64 and chunk
        #  2c+1 in partitions 64:128 at cols c*KC; q2 [128, HALF] holds q
        #  duplicated in both partition halves. Score matmuls then run as two
        #  concurrent K=64 row-group matmuls.
        v_sb = persist.tile([QT, N // QT, C], F16)
        if True:
            k2 = persist.tile([2 * C, N // 2], F32)
            q2 = persist.tile([2 * C, HALF], F32)
            def qkv_trio(dst, whl, i, h):
                xc = slice(i * KC, (i + 1) * KC)
                wtrio = [(whl[:, 0, :], xnh), (whl[:, 0, :], xnl),
                         (whl[:, 1, :], xnh)]
                for mi, (w, xs) in enumerate(wtrio):
                    nc.tensor.matmul(dst, lhsT=w, rhs=xs[:, xc],
                                     start=(mi == 0), stop=(mi == 2),
                                     tile_position=(0, h * C),
                                     skip_group_check=True)
            for i in range(N // KC):
                h = i % 2
                kp = gp.tile([2 * C, KC], F32, tag="gp", name=f"kp{i}")
                qkv_trio(kp[h * C:(h + 1) * C, :], wkhl, i, h)
                nc.scalar.copy(k2[h * C:(h + 1) * C, (i // 2) * KC:(i // 2 + 1) * KC],
                               kp[h * C:(h + 1) * C, :])
            for i in range(HALF // KC):
                for h in range(2):
                    qp = gp.tile([2 * C, KC], F32, tag="gp", name=f"qp{i}_{h}")
                    qkv_trio(qp[h * C:(h + 1) * C, :], wqhl, i, h)
                    nc.vector.tensor_copy(q2[h * C:(h + 1) * C, i * KC:(i + 1) * KC],
                                          qp[h * C:(h + 1) * C, :])
        for j in range(N // QT):
            vp = gp.tile([QT, C], F32, tag="gp", name=f"vp{j}")
            nc.tensor.matmul(vp, lhsT=xn1[:, j * QT:(j + 1) * QT], rhs=wv_sb,
                             start=True, stop=True)
            nc.vector.tensor_copy(v_sb[:, j, :], vp)

        # residual + output bias, for our query half
        xnb = persist.tile([C, HALF], F32)
        nc.vector.tensor_scalar_add(xnb, xn1[0:C, 0:HALF], bo_sb)

        # hi/lo bf16 split of q2/k2 for fast near-fp32 score matmuls:
        # s = qh*kh + qh*kl + ql*kh (ql*kl dropped, ~2e-4 abs error on scores)
        q2h = persist.tile([2 * C, HALF], BF16)
        q2l = persist.tile([2 * C, HALF], BF16)
        k2h = persist.tile([2 * C, N // 2], BF16)
        k2l = persist.tile([2 * C, N // 2], BF16)
        nc.vector.tensor_copy(q2h, q2)
        nc.vector.tensor_sub(q2l, q2, q2h)
        nc.vector.tensor_copy(k2h, k2)
        nc.vector.tensor_sub(k2l, k2, k2h)

        sums_all = persist.tile([QT, NQT], F32)
        attn_out = persist.tile([C, HALF], F16)
        fpre = persist.tile([C, HALF], F32)

        # ---- main loop over query tiles ----
        # Iteration t emits: score matmuls for tile t, INTERLEAVED with the
        # PE-mode transposes of tile t-1's attn. Interleaving keeps real
        # (HAM-counted) matmuls recurring within every ~3.4us window so the
        # PE clock stays at 2.4GHz — transpose-mode doesn't count as PE-busy
        # and a contiguous transpose burst re-throttles the clock to 1.2GHz.
        attnT = {}
        attn = {}
        for t in range(NQT + 1):
            if t < NQT:
                sca = sp.tile([QT, 2, KC], F32, tag="sc", name=f"sca{t}")
                scb = sp.tile([QT, 2, KC], F32, tag="sc", name=f"scb{t}")
                scc = sp.tile([QT, 2, KC], F32, tag="sc", name=f"scc{t}")
                park = park_pool.tile([QT, 2 * KC], F32, tag="park",
                                      name=f"park{t}")
                sc_tiles = [sca, scb, scc]
                cm = small.tile([QT, 4], F32, tag="cm", name=f"cm{t}")
                if t % GRP == 0:
                    attnT[t // GRP] = attnT_pool.tile(
                        [QT, N // QT, GRP * QT], F16, tag="attnT",
                        name=f"attnT{t}")

            otp = None
            if t >= 2 and t % 2 == 0:
                # attn@v for group g, interleaved below batch-by-batch
                g_av = t // 2 - 1
                otp = gp.tile([C, GRP * QT], F32, tag="gp", name=f"otp{g_av}")

            for cpos in range(NKC // 2):
                # Score pairs emitted in order [park, 0, 1, 2]: the parked
                # pair needs no score-PSUM slot, so it gives the PE work
                # during the window where exp(t-1) is still freeing slots.
                ci = (cpos + 3) % 4
                if t < NQT:
                    # score chunk pair ci as hi/lo bf16 trios; lanes h=0/1
                    # run concurrently in the two PE row groups (K=64 each),
                    # emitted interleaved so the lanes overlap.
                    tq = slice(t * QT, (t + 1) * QT)
                    kc = slice(ci * KC, (ci + 1) * KC)
                    trio = [(q2h, k2h), (q2h, k2l), (q2l, k2h)]
                    if ci < 3:
                        dsts = [sc_tiles[ci][:, 0, :], sc_tiles[ci][:, 1, :]]
                        for mi, (lo, ro) in enumerate(trio):
                            for h in range(2):
                                hs = slice(h * C, (h + 1) * C)
                                nc.tensor.matmul(dsts[h], lhsT=lo[hs, tq],
                                                 rhs=ro[hs, kc],
                                                 start=(mi == 0),
                                                 stop=(mi == 2),
                                                 skip_group_check=True)
                        nc.vector.tensor_reduce(cm[:, ci:ci + 1], sc_tiles[ci],
                                                axis=mybir.AxisListType.XY,
                                                op=mybir.AluOpType.max)
                    else:
                        # parked pair: lanes serial, sharing one gp slot
                        for h in range(2):
                            hs = slice(h * C, (h + 1) * C)
                            pp = gp.tile([QT, KC], F32, tag="gp",
                                         name=f"pp{t}_{h}")
                            for mi, (lo, ro) in enumerate(trio):
                                nc.tensor.matmul(pp, lhsT=lo[hs, tq],
                                                 rhs=ro[hs, kc],
                                                 start=(mi == 0),
                                                 stop=(mi == 2))
                            nc.vector.tensor_copy(
                                park[:, h * KC:(h + 1) * KC], pp)
                        nc.vector.tensor_reduce(cm[:, 3:4], park,
                                                axis=mybir.AxisListType.X,
                                                op=mybir.AluOpType.max)
                if t >= 1:
                    # transpose batch cpos of tile t-1 (ready once exp op
                    # cpos of tile t-1 has produced those attn columns)
                    tprev = t - 1
                    at = attn[tprev]
                    ag = attnT[tprev // GRP]
                    tp = gp.tile([QT, 8, QT], F16, tag="gp",
                                 name=f"tp{tprev}_{cpos}")
                    for c8 in range(8):
                        c = cpos * 8 + c8
                        nc.tensor.transpose(tp[:, c8, :],
                                            at[:, c * QT:(c + 1) * QT], ident)
                    dst = ag[:, cpos * 8:(cpos + 1) * 8,
                             (tprev % GRP) * QT:(tprev % GRP + 1) * QT]
                    if cpos % 2 == 0:
                        nc.vector.tensor_copy(dst, tp)
                    else:
                        nc.scalar.copy(dst, tp)
                if otp is not None:
                    # attn@v chunks of batch cpos (attnT for both tiles of
                    # the group is complete for these chunks)
                    ag = attnT[t // 2 - 1]
                    for c in range(cpos * 8, (cpos + 1) * 8):
                        nc.tensor.matmul(otp, lhsT=v_sb[:, c, :],
                                         rhs=ag[:, c, :], start=(c == 0),
                                         stop=(c == N // QT - 1),
                                         skip_group_check=True)

            if otp is not None:
                g_av = t // 2 - 1
                nc.vector.tensor_copy(
                    attn_out[:, g_av * GRP * QT:(g_av + 1) * GRP * QT], otp)
                attnT.pop(g_av)

            if t >= 4 and t % 4 == 0:
                # early output projection for the 512-col chunk completed by
                # the last two attn@v groups; only normalization + residual
                # remain for the epilogue
                j = t // 4 - 1
                sl = slice(j * KC, (j + 1) * KC)
                fp = gp.tile([C, KC], F32, tag="gp", name=f"fp{j}")
                nc.tensor.matmul(fp, lhsT=wo_sb, rhs=attn_out[:, sl],
                                 start=True, stop=True)
                nc.vector.tensor_copy(fpre[:, sl], fp)

            if t < NQT:
                nm = small.tile([QT, 1], F32, tag="nm", name=f"nm{t}")
                nc.vector.tensor_reduce(nm, cm, axis=mybir.AxisListType.X,
                                        op=mybir.AluOpType.max, negate=True)
                attn_t = attn_pool.tile([QT, N], F16, tag="attn",
                                        name=f"attn{t}")
                attn[t] = attn_t
                asum = small.tile([QT, 4], F32, tag="asum", name=f"asum{t}")
                for i, sct in enumerate(sc_tiles):
                    nc.scalar.activation(out=attn_t[:, i * 1024:(i + 1) * 1024],
                                         in_=sct.rearrange("p a b -> p (a b)"),
                                         func=mybir.ActivationFunctionType.Exp,
                                         bias=nm, scale=1.0,
                                         accum_out=asum[:, i:i + 1])
                nc.scalar.activation(out=attn_t[:, 3072:4096], in_=park,
                                     func=mybir.ActivationFunctionType.Exp,
                                     bias=nm, scale=1.0, accum_out=asum[:, 3:4])
                nc.vector.tensor_reduce(sums_all[:, t:t + 1], asum,
                                        axis=mybir.AxisListType.X,
                                        op=mybir.AluOpType.add)
                if t >= 1:
                    attn.pop(t - 1)

        # ---- epilogue: softmax normalization + output projection ----
        inv_all = persist.tile([QT, NQT], F32)
        nc.vector.reciprocal(out=inv_all, in_=sums_all)
        # scr[t, q'] = inv_all[q', t]  -> flat scr is inv ordered by global q
        nc.gpsimd.dma_start(out=scr_d.ap().rearrange("a b -> b a"), in_=inv_all)
        bcast = persist.tile([C, HALF], F32)
        scr_ap = scr_d.ap()
        bcast_src = bass.AP(tensor=scr_ap.tensor, offset=scr_ap.offset,
                            ap=[[0, C], [1, HALF]])
        nc.gpsimd.dma_start(out=bcast, in_=bcast_src)

        final_sb = persist.tile([C, HALF], F32)
        for j in range(HALF // KC):
            sl = slice(j * KC, (j + 1) * KC)
            nc.vector.tensor_mul(final_sb[:, sl], fpre[:, sl], bcast[:, sl])
            nc.vector.tensor_add(final_sb[:, sl], final_sb[:, sl], xnb[:, sl])
            nc.sync.dma_start(out=out_d.ap()[:, sl], in_=final_sb[:, sl])


def prep_inputs(x, w_qkv, b_qkv, w_out, b_out):
    """Host-side slicing/packing into per-core input maps."""
    x = np.asarray(x, dtype=np.float32).reshape(B, C, N)
    w_qkv = np.asarray(w_qkv, dtype=np.float32)
    b_qkv = np.asarray(b_qkv, dtype=np.float32)
    w_out = np.asarray(w_out, dtype=np.float32)
    b_out = np.asarray(b_out, dtype=np.float32)

    s = float(C) ** 0.5  # reference multiplies scores by sqrt(C)
    wq1 = np.concatenate([s * w_qkv[0:C].T, s * b_qkv[None, 0:C]], axis=0)
    wk1 = np.concatenate([w_qkv[C:2 * C].T, b_qkv[None, C:2 * C]], axis=0)

    def hilo(w):  # [65, 64] -> [65, 2, 64] (hi, lo) such that hi+lo == w
        hi = (w.view(np.uint32) & 0xFFFF0000).view(np.float32)
        return np.stack([hi, w - hi], axis=1)

    wq1 = hilo(np.ascontiguousarray(wq1))
    wk1 = hilo(np.ascontiguousarray(wk1))
    wv1 = np.concatenate([w_qkv[2 * C:3 * C].T, b_qkv[None, 2 * C:3 * C]], axis=0)
    wo16 = np.ascontiguousarray(w_out.T).astype(np.float16)
    bo = np.ascontiguousarray(b_out[:, None])

    in_maps = []
    for j in range(NCORES):
        b, h = divmod(j, 2)
        xs = x[b]
        if h == 1:
            xs = np.concatenate([xs[:, HALF:], xs[:, :HALF]], axis=1)
        in_maps.append({
            "x": np.ascontiguousarray(xs),
            "wq1": np.ascontiguousarray(wq1),
            "wk1": np.ascontiguousarray(wk1),
            "wv1": np.ascontiguousarray(wv1),
            "wo16": wo16,
            "bo": bo,
        })
    return in_maps


def gather_output(results):
    out = np.empty((B, C, N), dtype=np.float32)
    for j in range(NCORES):
        b, h = divmod(j, 2)
        out[b][:, h * HALF:(h + 1) * HALF] = results[j]["out"]
    return out.reshape(B, C, H, W)


_NC_CACHE = {}


def get_nc():
    key = "v3"
    if key not in _NC_CACHE:
        _NC_CACHE[key] = build_nc()
    return _NC_CACHE[key]


def kernel(x, w_qkv, b_qkv, w_out, b_out):
    nc = get_nc()
    in_maps = prep_inputs(x, w_qkv, b_qkv, w_out, b_out)
    res = run_bass_kernel_spmd(nc, in_maps, list(range(NCORES)))
    return gather_output(res.results)



# revision 4
# speedup vs baseline: 1.0968x; 1.0968x over previous
"""Trainium2 Bass kernel for an AttentionBlock (InstanceNorm + single-head
spatial self-attention + projection + residual).

Full-input contract: kernel(**inputs) takes the complete tensors and returns
the complete output. Internally shards across 8 NeuronCores: data-parallel
over batch (B=4 -> 4 pairs of cores), sequence-parallel over the N=4096 query
positions within each sample (2 cores per sample, 2048 queries each).

All 8 cores run the *same* program; the query-half assignment is done by
rotating the spatial columns of x host-side (attention and instance-norm
statistics are invariant under column permutation).

v2 design (transposed-scores / flash-style):
  - All matmuls in f16 (hi/lo double for the QKV projections, single f16 for
    the score matmuls; validated ~2.6e-3 rel_l2 end to end).
  - pass-1 ([q,k] layout): single f16 matmul per (q-tile, k-chunk), DVE
    row-max -> per-query -max, PE-transposed into the bias row of the
    pass-2 moving operand.
  - pass-2 ([k,q] layout): K=65 matmul (64 channels + bias row carrying
    -rowmax) produces shifted scores directly in PSUM; ScalarE exp writes
    f16 attention blocks straight to SBUF in the layout attn@v consumes.
    No PE transposes of attention, no PSUM-evacuation copies.
  - attn@v: v is augmented with a ones column, so the softmax denominators
    fall out of the same accumulation for free; normalization is applied
    after the output projection (it commutes).
"""

import os
import sys
import numpy as np
from contextlib import ExitStack

for _p in ("/opt/trn_rl_repo", "/root/.axon_site/_ro/trn_rl_repo"):
    if os.path.isdir(_p) and _p not in sys.path:
        sys.path.append(_p)

from concourse import bass, bacc, tile, mybir, masks  # noqa: E402
from concourse.bass_utils import run_bass_kernel_spmd  # noqa: E402

F32 = mybir.dt.float32
F16 = mybir.dt.float16

B, C, H, W = 4, 64, 64, 64
N = H * W            # 4096 spatial positions (attention length)
HALF = N // 2        # queries per core
KT = 128             # pass-2 k-tile (partition dim of transposed scores)
NKT = N // KT        # 32 k-tiles
QC = 512             # q-chunk (PSUM bank free dim)
NQC = HALF // QC     # 4 q-chunks per core
QT = 128             # pass-1 q-tile
KC = 512             # pass-1 k-chunk
NKC = N // KC        # 8
EPS = 1e-5
NCORES = 8


def build_nc():
    nc = bacc.Bacc("TRN2", target_bir_lowering=False, debug=False)

    x_d = nc.dram_tensor("x", [C, N], F32, kind="ExternalInput")
    wq_d = nc.dram_tensor("wq1", [C + 1, 2, C], F16, kind="ExternalInput")
    wk_d = nc.dram_tensor("wk1", [C + 1, 2, C], F16, kind="ExternalInput")
    wv_d = nc.dram_tensor("wv1", [C + 1, 2, C], F16, kind="ExternalInput")
    wo_d = nc.dram_tensor("wo16", [C, C], F16, kind="ExternalInput")
    bo_d = nc.dram_tensor("bo", [C, 1], F32, kind="ExternalInput")
    out_d = nc.dram_tensor("out", [C, HALF], F32, kind="ExternalOutput")

    with tile.TileContext(nc) as tc:
        _body(tc, x_d, wq_d, wk_d, wv_d, wo_d, bo_d, out_d)
    nc.compile()
    return nc


def _body(tc, x_d, wq_d, wk_d, wv_d, wo_d, bo_d, out_d):
    nc = tc.nc
    with ExitStack() as ctx:
        persist = ctx.enter_context(tc.tile_pool(name="persist", bufs=1))
        small = ctx.enter_context(tc.tile_pool(name="small", bufs=4))
        apool = ctx.enter_context(tc.tile_pool(name="apool", bufs=4))
        fpool = ctx.enter_context(tc.tile_pool(name="fpool", bufs=2))
        # PSUM: 3 + 3 + 2 banks
        p1p = ctx.enter_context(tc.tile_pool(name="p1p", bufs=3, space="PSUM"))
        scp = ctx.enter_context(tc.tile_pool(name="scp", bufs=3, space="PSUM"))
        avp = ctx.enter_context(tc.tile_pool(name="avp", bufs=2, space="PSUM"))

        # ---- inputs ----
        x_sb = persist.tile([C, N], F32)
        for i in range(8):
            sl = slice(i * (N // 8), (i + 1) * (N // 8))
            eng = nc.sync if i % 2 == 0 else nc.scalar
            eng.dma_start(out=x_sb[:, sl], in_=x_d.ap()[:, sl])
        wq_sb = persist.tile([C + 1, 2, C], F16)
        nc.sync.dma_start(out=wq_sb, in_=wq_d.ap())
        wk_sb = persist.tile([C + 1, 2, C], F16)
        nc.scalar.dma_start(out=wk_sb, in_=wk_d.ap())
        wv_sb = persist.tile([C + 1, 2, C], F16)
        nc.sync.dma_start(out=wv_sb, in_=wv_d.ap())
        wo_sb = persist.tile([C, C], F16)
        nc.scalar.dma_start(out=wo_sb, in_=wo_d.ap())
        bo_sb = persist.tile([C, 1], F32)
        nc.sync.dma_start(out=bo_sb, in_=bo_d.ap())
        eps_t = persist.tile([C, 1], F32)
        nc.vector.memset(eps_t, EPS)
        ones16 = persist.tile([1, C], F16)
        nc.gpsimd.memset(ones16, 1.0)
        ident = persist.tile([QT, QT], F16)
        masks.make_identity(nc, ident)

        # ---- instance norm stats ----
        stats = persist.tile([C, NKC, nc.vector.BN_STATS_DIM], F32)
        for i in range(NKC):
            nc.vector.bn_stats(out=stats[:, i, :], in_=x_sb[:, i * KC:(i + 1) * KC])
        mv = persist.tile([C, nc.vector.BN_AGGR_DIM], F32)
        nc.vector.bn_aggr(out=mv, in_=stats)
        stdv = persist.tile([C, 1], F32)
        nc.scalar.activation(out=stdv, in_=mv[:, 1:2],
                             func=mybir.ActivationFunctionType.Sqrt,
                             bias=eps_t, scale=1.0)
        rstd = persist.tile([C, 1], F32)
        nc.vector.reciprocal(out=rstd, in_=stdv)
        nmr = persist.tile([C, 1], F32)
        nc.vector.tensor_mul(nmr, mv[:, 0:1], rstd)
        nc.vector.tensor_scalar_mul(nmr, nmr, -1.0)

        # xn (f32, for residual + lo-part), f16 hi/lo with bias-row for QKV
        xn = persist.tile([C, N], F32)
        xnh = persist.tile([C + 1, N], F16)
        xnl = persist.tile([C + 1, N], F16)
        nc.gpsimd.memset(xnh[C:C + 1, :], 1.0)
        nc.gpsimd.memset(xnl[C:C + 1, :], 0.0)
        for i in range(4):
            hl = slice(i * (N // 4), (i + 1) * (N // 4))
            nc.scalar.activation(out=xn[:, hl], in_=x_sb[:, hl],
                                 func=mybir.ActivationFunctionType.Identity,
                                 bias=nmr, scale=rstd)
            nc.vector.tensor_copy(xnh[0:C, hl], xn[:, hl])
            nc.vector.tensor_sub(xnl[0:C, hl], xn[:, hl], xnh[0:C, hl])
        # residual + output bias for our query half
        xnb = persist.tile([C, HALF], F32)
        nc.vector.tensor_scalar_add(xnb, xn[:, 0:HALF], bo_sb)

        # ---- QKV projections (f16 double: wh@xh + wh@xl) ----
        kst = persist.tile([C + 1, N], F16)    # rows 0:64 k, row 64 ones
        qrhs = persist.tile([C + 1, HALF], F16)  # rows 0:64 q*sqrt(C), row 64 -max
        nc.gpsimd.memset(kst[C:C + 1, :], 1.0)
        vst = persist.tile([KT, NKT, 66], F16)   # [kpos, ktile, 64 v + ones + pad]
        nc.gpsimd.memset(vst[:, :, 64:65], 1.0)
        nc.gpsimd.memset(vst[:, :, 65:66], 0.0)

        for i in range(NKC):
            sl = slice(i * KC, (i + 1) * KC)
            kp = scp.tile([KT, KC], F32, tag="sc", name=f"kp{i}")
            nc.tensor.matmul(kp[0:C, :], lhsT=wk_sb[:, 0, :], rhs=xnh[:, sl],
                             start=True, stop=False, skip_group_check=True)
            nc.tensor.matmul(kp[0:C, :], lhsT=wk_sb[:, 0, :], rhs=xnl[:, sl],
                             start=False, stop=True, skip_group_check=True)
            if i % 2 == 0:
                nc.vector.tensor_copy(kst[0:C, sl], kp[0:C, :])
            else:
                nc.scalar.copy(kst[0:C, sl], kp[0:C, :])
        for i in range(NQC):
            sl = slice(i * QC, (i + 1) * QC)
            qp = scp.tile([KT, KC], F32, tag="sc", name=f"qp{i}")
            nc.tensor.matmul(qp[0:C, :], lhsT=wq_sb[:, 0, :], rhs=xnh[:, sl],
                             start=True, stop=False, skip_group_check=True)
            nc.tensor.matmul(qp[0:C, :], lhsT=wq_sb[:, 0, :], rhs=xnl[:, sl],
                             start=False, stop=True, skip_group_check=True)
            if i % 2 == 0:
                nc.vector.tensor_copy(qrhs[0:C, sl], qp[0:C, :])
            else:
                nc.scalar.copy(qrhs[0:C, sl], qp[0:C, :])
        for j in range(NKT):
            js = slice(j * KT, (j + 1) * KT)
            vp = p1p.tile([KT, C], F32, tag="p1", name=f"vp{j}")
            nc.tensor.matmul(vp, lhsT=xnh[:, js], rhs=wv_sb[:, 0, :],
                             start=True, stop=False, skip_group_check=True)
            nc.tensor.matmul(vp, lhsT=xnl[:, js], rhs=wv_sb[:, 0, :],
                             start=False, stop=True, skip_group_check=True)
            if j % 2 == 0:
                nc.vector.tensor_copy(vst[:, j, 0:C], vp)
            else:
                nc.scalar.copy(vst[:, j, 0:C], vp)

        # ---- pass-1 (row max) for one q-tile ----
        def pass1_tile(t):
            tq = slice(t * QT, (t + 1) * QT)
            cm = small.tile([QT, NKC], F32, tag="cm", name=f"cm{t}")
            for ci in range(NKC):
                cs = slice(ci * KC, (ci + 1) * KC)
                p1 = p1p.tile([QT, KC], F32, tag="p1", name=f"p1_{t}_{ci}")
                nc.tensor.matmul(p1, lhsT=qrhs[0:C, tq], rhs=kst[0:C, cs],
                                 start=True, stop=True, skip_group_check=True)
                nc.vector.tensor_reduce(cm[:, ci:ci + 1], p1,
                                        axis=mybir.AxisListType.X,
                                        op=mybir.AluOpType.max)
            nmT = small.tile([QT, C + 1], F16, tag="nmT", name=f"nmT{t}")
            nc.vector.tensor_reduce(nmT[:, C:C + 1], cm,
                                    axis=mybir.AxisListType.X,
                                    op=mybir.AluOpType.max, negate=True)
            tr = p1p.tile([C + 1, QT], F16, tag="p1", name=f"tr{t}")
            nc.tensor.transpose(tr, nmT, ident)
            nc.scalar.copy(qrhs[C:C + 1, tq], tr[C:C + 1, :])

        for t in range(4):
            pass1_tile(t)

        # ---- main loop: pass-2 + attn@v for chunk c, pass-1 for chunk c+1 ----
        ao16 = persist.tile([C, HALF], F16)
        inv16 = persist.tile([1, HALF], F16)
        for c in range(NQC):
            qs = slice(c * QC, (c + 1) * QC)
            otp = avp.tile([66, QC], F32, tag="av", name=f"otp{c}")
            for j in range(NKT):
                js = slice(j * KT, (j + 1) * KT)
                sc = scp.tile([KT, QC], F32, tag="sc", name=f"sc{c}_{j}")
                nc.tensor.matmul(sc, lhsT=kst[:, js], rhs=qrhs[:, qs],
                                 start=True, stop=True, skip_group_check=True)
                ab = apool.tile([KT, QC], F16, tag="ab", name=f"ab{c}_{j}")
                nc.scalar.activation(out=ab, in_=sc,
                                     func=mybir.ActivationFunctionType.Exp,
                                     bias=0.0, scale=1.0)
                nc.tensor.matmul(otp, lhsT=vst[:, j, :], rhs=ab,
                                 start=(j == 0), stop=(j == NKT - 1),
                                 skip_group_check=True)
                # interleave next chunk's pass-1, one tile per 8 k-tiles
                # (at j%8==1 so the last tile's DVE maxes drain before the
                # chunk boundary)
                if c + 1 < NQC and j % 8 == 1:
                    pass1_tile((c + 1) * 4 + j // 8)

            # epilogue for chunk c
            nc.scalar.copy(ao16[:, qs], otp[0:C, :])
            with nc.allow_low_precision(reason="1/sum fits f16; rel tol 2e-2"):
                nc.vector.reciprocal(out=inv16[:, qs], in_=otp[C:C + 1, :])
            fx = p1p.tile([KT, QC], F32, tag="p1", name=f"fx{c}")
            nc.tensor.matmul(fx[0:C, :], lhsT=wo_sb, rhs=ao16[:, qs],
                             start=True, stop=True, skip_group_check=True)
            fx2 = p1p.tile([KT, QC], F32, tag="p1", name=f"fx2{c}")
            nc.tensor.matmul(fx2[0:C, :], lhsT=ones16, rhs=inv16[:, qs],
                             start=True, stop=True, skip_group_check=True)
            ibs = fpool.tile([C, QC], F32, tag="ibs", name=f"ibs{c}")
            nc.scalar.copy(ibs, fx2[0:C, :])
            fin = fpool.tile([C, QC], F32, tag="fin", name=f"fin{c}")
            nc.vector.tensor_mul(fin, fx[0:C, :], ibs)
            nc.vector.tensor_add(fin, fin, xnb[:, qs])
            eng = nc.sync if c % 2 == 0 else nc.scalar
            eng.dma_start(out=out_d.ap()[:, qs], in_=fin)


def prep_inputs(x, w_qkv, b_qkv, w_out, b_out):
    """Host-side slicing/packing into per-core input maps."""
    x = np.asarray(x, dtype=np.float32).reshape(B, C, N)
    w_qkv = np.asarray(w_qkv, dtype=np.float32)
    b_qkv = np.asarray(b_qkv, dtype=np.float32)
    w_out = np.asarray(w_out, dtype=np.float32)
    b_out = np.asarray(b_out, dtype=np.float32)

    s = float(C) ** 0.5  # reference multiplies scores by sqrt(C)
    wq1 = np.concatenate([s * w_qkv[0:C].T, s * b_qkv[None, 0:C]], axis=0)
    wk1 = np.concatenate([w_qkv[C:2 * C].T, b_qkv[None, C:2 * C]], axis=0)
    wv1 = np.concatenate([w_qkv[2 * C:3 * C].T, b_qkv[None, 2 * C:3 * C]], axis=0)

    def hilo16(w):  # [65, 64] -> [65, 2, 64] f16 (hi, lo), hi+lo ~== w
        hi = w.astype(np.float16)
        lo = (w - hi.astype(np.float32)).astype(np.float16)
        return np.ascontiguousarray(np.stack([hi, lo], axis=1))

    wq1 = hilo16(np.ascontiguousarray(wq1))
    wk1 = hilo16(np.ascontiguousarray(wk1))
    wv1 = hilo16(np.ascontiguousarray(wv1))
    wo16 = np.ascontiguousarray(w_out.T).astype(np.float16)
    bo = np.ascontiguousarray(b_out[:, None])

    in_maps = []
    for j in range(NCORES):
        b, h = divmod(j, 2)
        xs = x[b]
        if h == 1:
            xs = np.concatenate([xs[:, HALF:], xs[:, :HALF]], axis=1)
        in_maps.append({
            "x": np.ascontiguousarray(xs),
            "wq1": wq1,
            "wk1": wk1,
            "wv1": wv1,
            "wo16": wo16,
            "bo": bo,
        })
    return in_maps


def gather_output(results):
    out = np.empty((B, C, N), dtype=np.float32)
    for j in range(NCORES):
        b, h = divmod(j, 2)
        out[b][:, h * HALF:(h + 1) * HALF] = results[j]["out"]
    return out.reshape(B, C, H, W)


_NC_CACHE = {}


def get_nc():
    key = "v2"
    if key not in _NC_CACHE:
        _NC_CACHE[key] = build_nc()
    return _NC_CACHE[key]


def kernel(x, w_qkv, b_qkv, w_out, b_out):
    nc = get_nc()
    in_maps = prep_inputs(x, w_qkv, b_qkv, w_out, b_out)
    res = run_bass_kernel_spmd(nc, in_maps, list(range(NCORES)))
    return gather_output(res.results)
